# revision 7
# baseline (speedup 1.0000x reference)
"""Trainium2 Bass kernel for nn_Attention3D (B=4, C=256, D=H=W=16).

y = x + wp @ softmax_j((wq@x+bq)^T (wk@x) / sqrt(C)) applied to (wv@x+bv), + bp

Sharding: 8 cores = (batch b, query-half). Each core owns one batch's full
K/V (N=4096 keys) and half the queries (2048). Key order is permuted per
core so "my" queries are always columns 0:2048 — softmax/attention are
invariant to key permutation, so every core runs the identical program.

Math folds (host side):
  - k-bias dropped: (q+bq)·(k+bk) = (q+bq)·k + [i-const] and softmax over
    keys j is invariant to per-query constants, so bk never matters.
  - v/out-projection fused: wvp = (wp@wv)^T applied key-major, so the
    attention matmul emits the projected output directly.
  - v-bias + out-bias folded into the epilogue: sum_j (pv_j + bvp) e_j
    = out_un + bvp*S, so y = out_un/S + (bvp + bp) + x.

Pipeline (per core):
  x arrives in 8 column pieces; projections (bf16, fp32 PSUM) are emitted
  per piece so PE starts ~3us in, and chunk-0 attention interleaves with
  the tail pieces. Attention runs fp8 e4m3 DoubleRow (contraction 256,
  2 MACs/cycle). exp on ScalarE (scale 1/16, output scaled 2^-4). The
  softmax denominator is accumulated as elementwise partial sums on
  DVE/GpSimd (bf16 accumulators) and folded across partitions by bf16
  ones-matmuls at the end of each chunk; epilogue y = out_un*(1/S) +
  (bp+bvp) + x streams per chunk from PSUM (residual read from bf16 x).
"""

import numpy as np
import ml_dtypes

B, C = 4, 256
D = H = W = 16
N = D * H * W          # 4096 voxels
P = 128                # partitions
CB = C // P            # 2 channel blocks
NI = N // 2            # 2048 queries per core
NCORES = 8
IC = 512               # i-chunk (one PSUM bank of fp32)
NIC = NI // IC         # 4 i-chunks
NJ = N // P            # 32 key blocks
NJ2 = NJ // 2          # 16 key superblocks (256 keys each, fp8 DoubleRow)
NP = 8                 # x pieces (512 cols each)
PC = N // NP           # piece cols
ESHIFT = -4 * 0.6931471805599453  # exp bias: fold 2^-4 so e fits fp8 e4m3

# ---- engine assignment config ----
# softmax denominator: in chunk 0 (which overlaps the projection pieces and
# must leave PSUM to them) partial sums go to two SBUF accumulators — DVE
# takes odd superblocks, GpSimd even ones; in chunks 1-3 odd superblocks
# accumulate on DVE and the rest ride TensorE as trailing fp8 DoubleRow
# ones-matmuls (GpSimd can't reach PSUM).
# exp: ScalarE real exp, except Schraudolph (DVE affine from PSUM + GpSimd
# clamp/convert in SBUF) for these superblocks:
EXP_SCHR_SBS = {1: (6, 12), 2: (6, 12), 3: (6, 12)}
# Schraudolph constants: u8 = clamp(0.72134752*s_psum + 24, 0, 119)
SCHR_A = 8 * 1.4426950408889634 / 16
SCHR_B = 24.0
SCHR_TOP = 119.0

_cache = {}


def _build():
    import concourse.bacc as bacc
    import concourse.mybir as mybir
    import concourse.tile as tile

    dt = mybir.dt
    f32, bf16, f8, u8 = dt.float32, dt.bfloat16, dt.float8e4, dt.uint8

    nc = bacc.Bacc("TRN2", target_bir_lowering=False, debug=False)

    x16_d = nc.dram_tensor("x16", [C, N], bf16, kind="ExternalInput")
    w_d = {
        w: nc.dram_tensor(w, [C, C], bf16, kind="ExternalInput")
        for w in ("wqT", "wkT", "wvpT")
    }
    bq_d = nc.dram_tensor("bq", [C, 1], f32, kind="ExternalInput")
    bpv_d = nc.dram_tensor("bpv", [C, 1], f32, kind="ExternalInput")
    y_d = nc.dram_tensor("y", [C, NI], f32, kind="ExternalOutput")

    add = mybir.AluOpType.add
    mx, mn, mult = (mybir.AluOpType.max, mybir.AluOpType.min,
                    mybir.AluOpType.mult)
    EXP = mybir.ActivationFunctionType.Exp
    DR = mybir.MatmulPerfMode.DoubleRow

    with tile.TileContext(nc) as tc:
        with (
            tc.tile_pool(name="consts", bufs=1) as consts,
            tc.tile_pool(name="acts", bufs=1) as acts,
            tc.tile_pool(name="e16p", bufs=20) as e16p,
            tc.tile_pool(name="sacc", bufs=4) as saccp,
            tc.tile_pool(name="schr", bufs=2) as schrp,
            tc.tile_pool(name="small", bufs=2) as small,
            tc.tile_pool(name="ys", bufs=3) as ys,
            tc.tile_pool(name="ps_s", bufs=2, space="PSUM") as ps_s,   # 4 banks
            tc.tile_pool(name="ps_a", bufs=2, space="PSUM") as ps_a,   # 2 banks
            tc.tile_pool(name="ps_v", bufs=2, space="PSUM") as ps_v,   # 2 banks
        ):
            # ---- constants ----
            bias_sb = {}
            for bname, bd in (("bq", bq_d), ("bpv", bpv_d)):
                bias_sb[bname] = []
                for cb in range(CB):
                    t = consts.tile([P, 1], f32, tag=f"{bname}{cb}", name=f"{bname}{cb}")
                    nc.gpsimd.dma_start(out=t, in_=bd.ap()[cb * P:(cb + 1) * P, :])
                    bias_sb[bname].append(t)
            ones16_t = consts.tile([P, P], bf16, tag="ones16")
            nc.vector.memset(ones16_t, 1.0)
            ones8_t = consts.tile([P, 2, P], f8, tag="ones8")
            nc.vector.memset(ones8_t, 1.0)
            eshift_t = consts.tile([P, 1], f32, tag="eshift")
            nc.vector.memset(eshift_t, ESHIFT)

            # ---- weights, then x pieces (both c-blocks per piece adjacent) ----
            w_sb = {w: [] for w in ("wqT", "wkT", "wvpT")}
            for wname in ("wqT", "wkT", "wvpT"):
                for cb in range(CB):
                    t = consts.tile([P, C], bf16, tag=f"{wname}{cb}", name=f"{wname}{cb}")
                    eng = nc.sync if cb == 0 else nc.gpsimd
                    eng.dma_start(out=t, in_=w_d[wname].ap()[cb * P:(cb + 1) * P, :])
                    w_sb[wname].append(t)
            xp = [[None] * NP for _ in range(CB)]
            for p in range(NP):
                for cb in range(CB):
                    t = acts.tile([P, PC], bf16, tag=f"x{cb}_{p}", name=f"x{cb}_{p}")
                    eng = nc.sync if cb == 0 else nc.gpsimd
                    eng.dma_start(out=t, in_=x16_d.ap()[cb * P:(cb + 1) * P,
                                                        p * PC:(p + 1) * PC])
                    xp[cb][p] = t

            # fp8 activations, pair-interleaved (channel c = pair*128 + ci)
            q16 = acts.tile([P, CB, NI], f8, tag="q16")       # [ci, pair, i]
            k16 = acts.tile([P, CB, N], f8, tag="k16")        # [ci, pair, j]
            vT16 = acts.tile([P, NJ2, 2, C], f8, tag="vT16")  # [ji, sb, pair, c]

            state = {}

            def emit_qproj(p):
                for ob in range(CB):
                    ps = ps_v.tile([P, IC], f32, tag="ps_v", name="qps")
                    for cb in range(CB):
                        nc.tensor.matmul(
                            ps, w_sb["wqT"][cb][:, ob * P:(ob + 1) * P], xp[cb][p],
                            start=(cb == 0), stop=(cb == CB - 1))
                    nc.vector.tensor_scalar_add(
                        q16[:, ob, p * PC:(p + 1) * PC], ps, bias_sb["bq"][ob])

            def emit_kproj(p):
                ps = ps_s.tile([P, CB, IC], f32, tag="ps_s", name="kps")
                for ob in range(CB):
                    for cb in range(CB):
                        nc.tensor.matmul(
                            ps[:, ob, :], w_sb["wkT"][cb][:, ob * P:(ob + 1) * P],
                            xp[cb][p],
                            start=(cb == 0), stop=(cb == CB - 1))
                nc.scalar.copy(k16[:, :, p * PC:(p + 1) * PC], ps)

            def emit_vproj(p, jp):
                # j pair (4p+2jp, 4p+2jp+1) -> superblock sb = j0//2, both pairs
                j0 = 4 * p + 2 * jp
                ps = ps_v.tile([P, 2, C], f32, tag="ps_v", name="vps")
                for r in range(2):
                    lb = (j0 + r - 4 * p) * P
                    for cb in range(CB):
                        nc.tensor.matmul(
                            ps[:, r, :], xp[cb][p][:, lb:lb + P], w_sb["wvpT"][cb],
                            start=(cb == 0), stop=(cb == CB - 1))
                nc.vector.tensor_copy(vT16[:, j0 // 2, :, :], ps)

            def sacc_accum(key, e16, eng):
                if key not in state:
                    t = saccp.tile([P, 2, IC], bf16, tag="sacc",
                                   name=f"sacc_{key[0]}{key[1]}")
                    eng.tensor_copy(t, e16)
                    state[key] = t
                else:
                    eng.tensor_add(state[key], state[key], e16)

            def emit_attn_sb(ic, sb):
                isl = slice(ic * IC, (ic + 1) * IC)
                s_ps = ps_s.tile([P, 2, IC], f32, tag="ps_s", name="sps")
                for r in range(2):
                    jb = 2 * sb + r
                    nc.tensor.matmul(
                        s_ps[:, r, :],
                        k16[:, :, jb * P:(jb + 1) * P], q16[:, :, isl],
                        start=True, stop=True, perf_mode=DR)
                e16 = e16p.tile([P, 2, IC], f8, tag="e16")
                if sb in EXP_SCHR_SBS.get(ic, ()):
                    # Schraudolph: affine in log2 space lands the e4m3 bit
                    # pattern directly; DVE does the PSUM-side affine, GpSimd
                    # the SBUF-side clamp + uint8 convert (RNE).
                    tmp = schrp.tile([P, 2, IC], f32, tag="schr")
                    nc.vector.tensor_scalar(out=tmp, in0=s_ps, scalar1=SCHR_A,
                                            scalar2=-SCHR_B, op0=mult, op1=mx)
                    nc.gpsimd.tensor_scalar(out=e16.bitcast(u8),
                                            in0=tmp, scalar1=SCHR_B,
                                            scalar2=SCHR_TOP, op0=add, op1=mn)
                else:
                    nc.scalar.activation(e16, s_ps, EXP,
                                         scale=float(C) ** -0.5, bias=eshift_t)
                first, last = (sb == 0), (sb == NJ2 - 1)
                for cb in range(CB):
                    nc.tensor.matmul(
                        state[("a_ps", ic)][cb],
                        vT16[:, sb, :, cb * P:(cb + 1) * P], e16,
                        start=first, stop=last, perf_mode=DR)
                # ---- denominator partial sums ----
                if ic == 0:
                    # PSUM belongs to the interleaved projections: SBUF-only
                    # accumulators (DVE odd sbs + last, GpSimd even)
                    if sb % 2 == 1 or sb == NJ2 - 1:
                        sacc_accum(("Sd", ic), e16, nc.vector)
                    else:
                        sacc_accum(("Sp", ic), e16, nc.gpsimd)
                    return
                # chunks 1-3: odd sbs (except last) on DVE, rest trail on
                # TensorE as DoubleRow ones-matmuls into S_ps
                if sb % 2 == 1 and sb < NJ2 - 1:
                    sacc_accum(("Sd", ic), e16, nc.vector)
                else:
                    epe = state.setdefault(("epe", ic), [])
                    epe.append(e16)
                    if len(epe) > 1:
                        nc.tensor.matmul(state[("S_ps", ic)], ones8_t,
                                         epe.pop(0), start=(sb == 2),
                                         stop=False, perf_mode=DR)
                if sb == NJ2 - 2:
                    # Sacc complete (last odd sb was 13): fold into S_ps now
                    for r in range(2):
                        nc.tensor.matmul(state[("S_ps", ic)], ones16_t,
                                         state[("Sd", ic)][:, r, :],
                                         start=False, stop=False)

            def emit_chunk_start(ic):
                state[("a_ps", ic)] = [
                    ps_a.tile([P, IC], f32, tag="ps_a", name=f"aps{ic}_{cb}")
                    for cb in range(CB)]
                if ic > 0:
                    state[("S_ps", ic)] = ps_v.tile([P, IC], f32, tag="ps_v",
                                                    name=f"S_ps{ic}")

            def emit_chunk_end(ic):
                isl = slice(ic * IC, (ic + 1) * IC)
                if ic == 0:
                    S_ps = ps_v.tile([P, IC], f32, tag="ps_v", name="S_ps0")
                    mms = [(state[k], r)
                           for k in (("Sd", ic), ("Sp", ic)) for r in range(2)]
                    for n_, (a, r) in enumerate(mms):
                        nc.tensor.matmul(S_ps, ones16_t, a[:, r, :],
                                         start=(n_ == 0),
                                         stop=(n_ == len(mms) - 1))
                else:
                    S_ps = state[("S_ps", ic)]
                    epe = state[("epe", ic)]
                    for n_, t in enumerate(epe):
                        nc.tensor.matmul(S_ps, ones8_t, t, start=False,
                                         stop=(n_ == len(epe) - 1),
                                         perf_mode=DR)
                R = small.tile([P, IC], f32, tag="R")
                nc.vector.reciprocal_approx_fast(out=R, in_=S_ps)
                for ob in range(CB):
                    tmp = ys.tile([P, IC], f32, tag="tmp")
                    nc.vector.tensor_mul(tmp, state[("a_ps", ic)][ob], R)
                    yt = ys.tile([P, IC], f32, tag="yt")
                    nc.vector.scalar_tensor_tensor(
                        yt, tmp, bias_sb["bpv"][ob], xp[ob][ic],
                        op0=add, op1=add)
                    nc.sync.dma_start(out=y_d.ap()[ob * P:(ob + 1) * P, isl], in_=yt)

            # ---- emission: projections per piece, chunk-0 attn interleaved ----
            emit_chunk_start(0)
            for p in range(NP):
                emit_kproj(p)
                if p < NIC:
                    emit_qproj(p)
                emit_vproj(p, 0)
                emit_vproj(p, 1)
                if p >= 1:
                    # sbs unlocked by previous piece (k/v of piece p-1)
                    emit_attn_sb(0, 2 * (p - 1))
                    emit_attn_sb(0, 2 * (p - 1) + 1)
            emit_attn_sb(0, 2 * (NP - 1))
            emit_attn_sb(0, 2 * (NP - 1) + 1)
            emit_chunk_end(0)
            for ic in range(1, NIC):
                emit_chunk_start(ic)
                for sb in range(NJ2):
                    emit_attn_sb(ic, sb)
                emit_chunk_end(ic)

    nc.compile()
    return nc


def _prep_inputs(x, wq, bq, wk, bk, wv, bv, wp, bp):
    bf16 = ml_dtypes.bfloat16
    xf = np.asarray(x, np.float32).reshape(B, C, N)
    wp64 = np.asarray(wp, np.float64)
    wv64 = np.asarray(wv, np.float64)
    shared = {
        "wqT": np.ascontiguousarray(np.asarray(wq, np.float32).T).astype(bf16),
        "wkT": np.ascontiguousarray(np.asarray(wk, np.float32).T).astype(bf16),
        # out-projection folded into the v-projection: wp @ (v·p) == (wvp^T x)·p
        "wvpT": np.ascontiguousarray((wp64 @ wv64).T.astype(np.float32)).astype(bf16),
        "bq": np.asarray(bq, np.float32).reshape(C, 1),
        # v-bias and out-bias both fold into the epilogue constant
        "bpv": (np.asarray(bp, np.float64)
                + wp64 @ np.asarray(bv, np.float64)).astype(np.float32).reshape(C, 1),
    }
    in_maps = []
    for core in range(NCORES):
        b, h = core // 2, core % 2
        xs = xf[b]
        if h == 1:  # roll so this core's query half is first (key order irrelevant)
            xs = np.concatenate([xs[:, NI:], xs[:, :NI]], axis=1)
        m = dict(shared)
        m["x16"] = np.ascontiguousarray(xs).astype(bf16)
        in_maps.append(m)
    return in_maps


def _run(inputs, trace=False, **kwargs):
    from concourse.bass_utils import run_bass_kernel_spmd

    if "nc" not in _cache:
        _cache["nc"] = _build()
    nc = _cache["nc"]
    in_maps = _prep_inputs(**inputs)
    res = run_bass_kernel_spmd(
        nc, in_maps, core_ids=list(range(NCORES)), trace=trace, **kwargs
    )
    out = np.empty((B, C, N), np.float32)
    for core in range(NCORES):
        b, h = core // 2, core % 2
        out[b][:, h * NI:(h + 1) * NI] = res.results[core]["y"]
    return out.reshape(B, C, D, H, W), res


def kernel(**inputs):
    out, _ = _run(inputs)
    return out


# revision 8
# speedup vs baseline: 1.8134x; 1.8134x over previous
"""Trainium2 Bass kernel for nn_Attention3D (B=4, C=256, D=H=W=16).

y = x + wp @ softmax_j((wq@x+bq)^T (wk@x) / sqrt(C)) applied to (wv@x+bv), + bp

Sharding: 8 cores = (batch b, query-half). Each core owns one batch's full
K/V (N=4096 keys) and half the queries (2048). Key order is permuted per
core so "my" queries are always columns 0:2048 — softmax/attention are
invariant to key permutation, so every core runs the identical program.

Math folds (host side):
  - k-bias dropped: (q+bq)·(k+bk) = (q+bq)·k + [i-const] and softmax over
    keys j is invariant to per-query constants, so bk never matters.
  - v/out-projection fused: wvp = (wp@wv)^T applied key-major, so the
    attention matmul emits the projected output directly.
  - v-bias + out-bias folded into the epilogue: sum_j (pv_j + bvp) e_j
    = out_un + bvp*S, so y = out_un/S + (bvp + bp) + x.

Pipeline (per core):
  All projections AND attention run fp8 e4m3 DoubleRow (contraction 256,
  2 MACs/cycle): x ships pre-pair-interleaved fp8 alongside a bf16 copy of
  the query columns for the residual. x8 arrives in 8 column pieces;
  projections are emitted per piece so PE starts ~3us in, and chunk-0
  attention interleaves with the tail pieces. exp on ScalarE (scale 1/16,
  output scaled 2^-4), except a few superblocks per chunk computed on DVE
  as a Schraudolph affine (log2-space affine -> e4m3 bit pattern). The
  softmax denominator: chunk 0 accumulates SBUF partial sums (DVE odd
  superblocks, GpSimd even — PSUM belongs to the projections then); chunks
  1-3 put odd superblocks on DVE and the rest trail on TensorE as fp8
  DoubleRow ones-matmuls. Epilogue y = out_un*(1/S) + (bp+bvp) + x.
"""

import numpy as np
import ml_dtypes

B, C = 4, 256
D = H = W = 16
N = D * H * W          # 4096 voxels
P = 128                # partitions
CB = C // P            # 2 channel blocks
NI = N // 2            # 2048 queries per core
NCORES = 8
IC = 512               # i-chunk (one PSUM bank of fp32)
NIC = NI // IC         # 4 i-chunks
NJ = N // P            # 32 key blocks
NJ2 = NJ // 2          # 16 key superblocks (256 keys each, fp8 DoubleRow)
NP = 8                 # x pieces (512 cols each)
PC = N // NP           # piece cols
ESHIFT = -4 * 0.6931471805599453  # exp bias: fold 2^-4 so e fits fp8 e4m3

# exp: ScalarE real exp, except Schraudolph on DVE (PSUM affine + SBUF
# clamp/convert, both DVE) for these superblocks:
EXP_SCHR_SBS = {1: (6, 12), 2: (6, 12), 3: (6, 12)}
# Schraudolph constants: u8 = clamp(0.72134752*s_psum + 24, 0, 119)
SCHR_A = 8 * 1.4426950408889634 / 16
SCHR_B = 24.0
SCHR_TOP = 119.0

_cache = {}


def _build():
    import concourse.bacc as bacc
    import concourse.mybir as mybir
    import concourse.tile as tile

    dt = mybir.dt
    f32, bf16, f8, u8 = dt.float32, dt.bfloat16, dt.float8e4, dt.uint8

    nc = bacc.Bacc("TRN2", target_bir_lowering=False, debug=False)

    # x8: pair-interleaved fp8 x8[ci, pair*N + n] = x[pair*128+ci, n]
    x8_d = nc.dram_tensor("x8", [P, CB * N], f8, kind="ExternalInput")
    # bf16 query columns for the residual add
    xq_d = nc.dram_tensor("xq", [C, NI], bf16, kind="ExternalInput")
    # weights pair-interleaved: w8[ci, pair*C + cout] = w[cout, pair*128+ci]
    w_d = {
        w: nc.dram_tensor(w, [P, CB * C], f8, kind="ExternalInput")
        for w in ("wq8", "wk8", "wv8")
    }
    bq_d = nc.dram_tensor("bq", [C, 1], f32, kind="ExternalInput")
    bpv_d = nc.dram_tensor("bpv", [C, 1], f32, kind="ExternalInput")
    y_d = nc.dram_tensor("y", [C, NI], f32, kind="ExternalOutput")

    add = mybir.AluOpType.add
    mx, mn, mult = (mybir.AluOpType.max, mybir.AluOpType.min,
                    mybir.AluOpType.mult)
    EXP = mybir.ActivationFunctionType.Exp
    DR = mybir.MatmulPerfMode.DoubleRow

    with tile.TileContext(nc) as tc:
        with (
            tc.tile_pool(name="consts", bufs=1) as consts,
            tc.tile_pool(name="acts", bufs=1) as acts,
            tc.tile_pool(name="e16p", bufs=20) as e16p,
            tc.tile_pool(name="sacc", bufs=4) as saccp,
            tc.tile_pool(name="schr", bufs=2) as schrp,
            tc.tile_pool(name="small", bufs=2) as small,
            tc.tile_pool(name="ys", bufs=3) as ys,
            tc.tile_pool(name="ps_s", bufs=2, space="PSUM") as ps_s,   # 4 banks
            tc.tile_pool(name="ps_a", bufs=2, space="PSUM") as ps_a,   # 2 banks
            tc.tile_pool(name="ps_v", bufs=2, space="PSUM") as ps_v,   # 2 banks
        ):
            # ---- constants ----
            bias_sb = {}
            for bname, bd in (("bq", bq_d), ("bpv", bpv_d)):
                bias_sb[bname] = []
                for cb in range(CB):
                    t = consts.tile([P, 1], f32, tag=f"{bname}{cb}", name=f"{bname}{cb}")
                    nc.gpsimd.dma_start(out=t, in_=bd.ap()[cb * P:(cb + 1) * P, :])
                    bias_sb[bname].append(t)
            ones16_t = consts.tile([P, P], bf16, tag="ones16")
            nc.vector.memset(ones16_t, 1.0)
            ones8_t = consts.tile([P, 2, P], f8, tag="ones8")
            nc.vector.memset(ones8_t, 1.0)
            eshift_t = consts.tile([P, 1], f32, tag="eshift")
            nc.vector.memset(eshift_t, ESHIFT)

            # ---- weights first, then x8 pieces (pair halves adjacent) ----
            w_sb = {}
            for n_, wname in enumerate(("wk8", "wq8", "wv8")):
                t = consts.tile([P, CB, C], f8, tag=wname, name=wname)
                eng = nc.gpsimd if wname == "wv8" else nc.sync
                for pair in range(CB):
                    eng.dma_start(out=t[:, pair, :],
                                  in_=w_d[wname].ap()[:, pair * C:(pair + 1) * C])
                w_sb[wname] = t
            xp = [None] * NP
            for p in range(NP):
                t = acts.tile([P, CB, PC], f8, tag=f"x8_{p}", name=f"x8_{p}")
                for pair in range(CB):
                    eng = nc.sync if pair == 0 else nc.gpsimd
                    eng.dma_start(
                        out=t[:, pair, :],
                        in_=x8_d.ap()[:, pair * N + p * PC:pair * N + (p + 1) * PC])
                xp[p] = t
            # residual (bf16), needed only at chunk ends
            xq = []
            for ob in range(CB):
                t = acts.tile([P, NI], bf16, tag=f"xq{ob}", name=f"xq{ob}")
                nc.sync.dma_start(out=t, in_=xq_d.ap()[ob * P:(ob + 1) * P, :])
                xq.append(t)

            # fp8 activations, pair-interleaved (channel c = pair*128 + ci)
            q16 = acts.tile([P, CB, NI], f8, tag="q16")       # [ci, pair, i]
            k16 = acts.tile([P, CB, N], f8, tag="k16")        # [ci, pair, j]
            vT16 = acts.tile([P, NJ2, 2, C], f8, tag="vT16")  # [ji, sb, pair, c]

            state = {}

            def emit_qproj(p):
                for ob in range(CB):
                    ps = ps_v.tile([P, IC], f32, tag="ps_v", name="qps")
                    nc.tensor.matmul(
                        ps, w_sb["wq8"][:, :, ob * P:(ob + 1) * P], xp[p],
                        start=True, stop=True, perf_mode=DR)
                    nc.scalar.add(q16[:, ob, p * PC:(p + 1) * PC], ps,
                                  bias_sb["bq"][ob])

            def emit_kproj(p):
                ps = ps_s.tile([P, CB, IC], f32, tag="ps_s", name="kps")
                for ob in range(CB):
                    nc.tensor.matmul(
                        ps[:, ob, :], w_sb["wk8"][:, :, ob * P:(ob + 1) * P],
                        xp[p], start=True, stop=True, perf_mode=DR)
                nc.scalar.copy(k16[:, :, p * PC:(p + 1) * PC], ps)

            def emit_vproj(p, jp):
                # j pair (4p+2jp, 4p+2jp+1) -> superblock sb = j0//2, both pairs
                j0 = 4 * p + 2 * jp
                ps = ps_v.tile([P, 2, C], f32, tag="ps_v", name="vps")
                for r in range(2):
                    lb = (j0 + r - 4 * p) * P
                    nc.tensor.matmul(
                        ps[:, r, :], xp[p][:, :, lb:lb + P], w_sb["wv8"],
                        start=True, stop=True, perf_mode=DR)
                nc.vector.tensor_copy(vT16[:, j0 // 2, :, :], ps)

            def sacc_accum(key, e16, eng):
                if key not in state:
                    t = saccp.tile([P, 2, IC], bf16, tag="sacc",
                                   name=f"sacc_{key[0]}{key[1]}")
                    eng.tensor_copy(t, e16)
                    state[key] = t
                else:
                    eng.tensor_add(state[key], state[key], e16)

            def emit_attn_sb(ic, sb):
                isl = slice(ic * IC, (ic + 1) * IC)
                s_ps = ps_s.tile([P, 2, IC], f32, tag="ps_s", name="sps")
                for r in range(2):
                    jb = 2 * sb + r
                    nc.tensor.matmul(
                        s_ps[:, r, :],
                        k16[:, :, jb * P:(jb + 1) * P], q16[:, :, isl],
                        start=True, stop=True, perf_mode=DR)
                e16 = e16p.tile([P, 2, IC], f8, tag="e16")
                if sb in EXP_SCHR_SBS.get(ic, ()):
                    # Schraudolph: affine in log2 space lands the e4m3 bit
                    # pattern directly (DVE both ops; the SBUF-side convert
                    # gets the single-source 2x port mode)
                    tmp = schrp.tile([P, 2, IC], f32, tag="schr")
                    nc.vector.tensor_scalar(out=tmp, in0=s_ps, scalar1=SCHR_A,
                                            scalar2=-SCHR_B, op0=mult, op1=mx)
                    nc.vector.tensor_scalar(out=e16.bitcast(u8), in0=tmp,
                                            scalar1=SCHR_B, scalar2=SCHR_TOP,
                                            op0=add, op1=mn)
                else:
                    nc.scalar.activation(e16, s_ps, EXP,
                                         scale=float(C) ** -0.5, bias=eshift_t)
                first, last = (sb == 0), (sb == NJ2 - 1)
                for cb in range(CB):
                    nc.tensor.matmul(
                        state[("a_ps", ic)][cb],
                        vT16[:, sb, :, cb * P:(cb + 1) * P], e16,
                        start=first, stop=last, perf_mode=DR)
                # ---- denominator partial sums ----
                if ic == 0:
                    # PSUM belongs to the interleaved projections: SBUF-only
                    # accumulators (DVE odd sbs + last, GpSimd even)
                    if sb % 2 == 1 or sb == NJ2 - 1:
                        sacc_accum(("Sd", ic), e16, nc.vector)
                    else:
                        sacc_accum(("Sp", ic), e16, nc.gpsimd)
                    return
                # chunks 1-3: odd sbs (except last) on DVE, rest trail on
                # TensorE as DoubleRow ones-matmuls into S_ps
                if sb % 2 == 1 and sb < NJ2 - 1:
                    sacc_accum(("Sd", ic), e16, nc.vector)
                else:
                    epe = state.setdefault(("epe", ic), [])
                    epe.append(e16)
                    if len(epe) > 1:
                        nc.tensor.matmul(state[("S_ps", ic)], ones8_t,
                                         epe.pop(0), start=(sb == 2),
                                         stop=False, perf_mode=DR)
                if sb == NJ2 - 2:
                    # Sacc complete (last odd sb was 13): fold into S_ps now
                    for r in range(2):
                        nc.tensor.matmul(state[("S_ps", ic)], ones16_t,
                                         state[("Sd", ic)][:, r, :],
                                         start=False, stop=False)

            def emit_chunk_start(ic):
                state[("a_ps", ic)] = [
                    ps_a.tile([P, IC], f32, tag="ps_a", name=f"aps{ic}_{cb}")
                    for cb in range(CB)]
                if ic > 0:
                    state[("S_ps", ic)] = ps_v.tile([P, IC], f32, tag="ps_v",
                                                    name=f"S_ps{ic}")

            def emit_chunk_end(ic):
                isl = slice(ic * IC, (ic + 1) * IC)
                if ic == 0:
                    S_ps = ps_v.tile([P, IC], f32, tag="ps_v", name="S_ps0")
                    mms = [(state[k], r)
                           for k in (("Sd", ic), ("Sp", ic)) for r in range(2)]
                    for n_, (a, r) in enumerate(mms):
                        nc.tensor.matmul(S_ps, ones16_t, a[:, r, :],
                                         start=(n_ == 0),
                                         stop=(n_ == len(mms) - 1))
                else:
                    S_ps = state[("S_ps", ic)]
                    epe = state[("epe", ic)]
                    for n_, t in enumerate(epe):
                        nc.tensor.matmul(S_ps, ones8_t, t, start=False,
                                         stop=(n_ == len(epe) - 1),
                                         perf_mode=DR)
                R = small.tile([P, IC], f32, tag="R")
                nc.vector.reciprocal_approx_fast(out=R, in_=S_ps)
                for ob in range(CB):
                    tmp = ys.tile([P, IC], f32, tag="tmp")
                    nc.vector.tensor_mul(tmp, state[("a_ps", ic)][ob], R)
                    yt = ys.tile([P, IC], f32, tag="yt")
                    nc.vector.scalar_tensor_tensor(
                        yt, tmp, bias_sb["bpv"][ob], xq[ob][:, isl],
                        op0=add, op1=add)
                    nc.sync.dma_start(out=y_d.ap()[ob * P:(ob + 1) * P, isl], in_=yt)

            # ---- emission: projections per piece, chunk-0 attn interleaved ----
            emit_chunk_start(0)
            for p in range(NP):
                emit_kproj(p)
                if p < NIC:
                    emit_qproj(p)
                emit_vproj(p, 0)
                emit_vproj(p, 1)
                if p >= 1:
                    # sbs unlocked by previous piece (k/v of piece p-1)
                    emit_attn_sb(0, 2 * (p - 1))
                    emit_attn_sb(0, 2 * (p - 1) + 1)
            emit_attn_sb(0, 2 * (NP - 1))
            emit_attn_sb(0, 2 * (NP - 1) + 1)
            emit_chunk_end(0)
            for ic in range(1, NIC):
                emit_chunk_start(ic)
                for sb in range(NJ2):
                    emit_attn_sb(ic, sb)
                emit_chunk_end(ic)

    nc.compile()
    return nc


def _prep_inputs(x, wq, bq, wk, bk, wv, bv, wp, bp):
    bf16 = ml_dtypes.bfloat16
    fp8 = ml_dtypes.float8_e4m3
    xf = np.asarray(x, np.float32).reshape(B, C, N)
    wp64 = np.asarray(wp, np.float64)
    wv64 = np.asarray(wv, np.float64)

    def interleave_w(w):  # [cout, cin] -> [ci, pair*C + cout] fp8
        wT = np.asarray(w, np.float32).T.reshape(CB, P, C)      # [pair, ci, cout]
        return np.ascontiguousarray(
            wT.transpose(1, 0, 2).reshape(P, CB * C)).astype(fp8)

    shared = {
        "wq8": interleave_w(wq),
        "wk8": interleave_w(wk),
        "wv8": interleave_w((wp64 @ wv64).astype(np.float32)),
        "bq": np.asarray(bq, np.float32).reshape(C, 1),
        "bpv": (np.asarray(bp, np.float64)
                + wp64 @ np.asarray(bv, np.float64)).astype(np.float32).reshape(C, 1),
    }
    in_maps = []
    for core in range(NCORES):
        b, h = core // 2, core % 2
        xs = xf[b]
        if h == 1:  # roll so this core's query half is first (key order irrelevant)
            xs = np.concatenate([xs[:, NI:], xs[:, :NI]], axis=1)
        m = dict(shared)
        x8 = xs.reshape(CB, P, N).transpose(1, 0, 2).reshape(P, CB * N)
        m["x8"] = np.ascontiguousarray(x8).astype(fp8)
        m["xq"] = np.ascontiguousarray(xs[:, :NI]).astype(bf16)
        in_maps.append(m)
    return in_maps


def _run(inputs, trace=False, **kwargs):
    from concourse.bass_utils import run_bass_kernel_spmd

    if "nc" not in _cache:
        _cache["nc"] = _build()
    nc = _cache["nc"]
    in_maps = _prep_inputs(**inputs)
    res = run_bass_kernel_spmd(
        nc, in_maps, core_ids=list(range(NCORES)), trace=trace, **kwargs
    )
    out = np.empty((B, C, N), np.float32)
    for core in range(NCORES):
        b, h = core // 2, core % 2
        out[b][:, h * NI:(h + 1) * NI] = res.results[core]["y"]
    return out.reshape(B, C, D, H, W), res


def kernel(**inputs):
    out, _ = _run(inputs)
    return out
